# revision 6
# baseline (speedup 1.0000x reference)
"""Trainium2 Bass kernel for CompressedAttentionWrapper.

Sharding: tensor-parallel over heads across 8 NeuronCores (2 heads/core).
Wq/Wk/Wv column-split by head; per-head compressors and compressed cache
local to each core. The output projection redistributes the per-head
attention context with an on-chip AllToAll so each core computes a 512-row
slice of the final output against the full Wo.

Per-core dataflow (all big matmuls in float32r at full PE rate):
  phase 1: stream X in 256-row chunks, PE-transpose to X^T, project to
           qT/kT/vT (dh on partitions), chain the K/V compression
           (k_c = k@Wkc) and expansion (k_hat = k_c@Wke) per head; emit
           k_compressed / v_compressed output tiles on the way.
  phase 2: causal attention per (batch, head) on transposed scores:
           scoresT = k_hatT.T-blocks @ qT, exp on ACT (scale fused), causal
           handled structurally (block skipping + additive diag mask),
           row-sums via ones-matmul, ctxT accumulated in PSUM, normalized
           with reciprocal broadcast.
  phase 3: AllToAll of normalized ctxT (4MB/core), then out-rows slice =
           gathered-ctxT.T @ full Wo.
"""

import sys

for _p in ("/opt/trn_rl_repo", "/opt/pypackages"):
    if _p not in sys.path:
        sys.path.append(_p)

import numpy as np
import concourse.bass as bass  # noqa: F401  (import keeps bass registered)
import concourse.mybir as mybir
import concourse.tile as tile
from concourse import bacc
from concourse.bass_utils import run_bass_kernel_spmd
from concourse.masks import make_identity

B, S, D = 2, 2048, 2048
H, DH, DC = 16, 128, 32
N_CORES = 8
HL = H // N_CORES           # heads per core (2)
E = HL * DH                 # local projection width (256)
R = B * S                   # flattened rows (4096)
RL = R // N_CORES           # output rows per core (512)
CH = 256                    # phase-1 row chunk
NCH = R // CH
QC = 512                    # phase-2 query chunk
NQC = S // QC
KB = D // 128               # contraction blocks (16)
SCALE = 1.0 / float(np.sqrt(DH))
MASK_VAL = -1.0e5

f32 = mybir.dt.float32
f32r = mybir.dt.float32r

_PROG = {}


def _build_program():
    nc = bacc.Bacc("TRN2", target_bir_lowering=False, debug=False,
                   num_devices=N_CORES)

    x = nc.dram_tensor("x", [R, D], f32, kind="ExternalInput")
    wq = nc.dram_tensor("wq", [D, E], f32r, kind="ExternalInput")
    wk = nc.dram_tensor("wk", [D, E], f32r, kind="ExternalInput")
    wv = nc.dram_tensor("wv", [D, E], f32r, kind="ExternalInput")
    wo = nc.dram_tensor("wo", [D, D], f32r, kind="ExternalInput")
    wkc = nc.dram_tensor("wkc", [DH, DC], f32r, kind="ExternalInput")
    wvc = nc.dram_tensor("wvc", [DH, DC], f32r, kind="ExternalInput")
    wke = nc.dram_tensor("wke", [DC, DH], f32r, kind="ExternalInput")
    wve = nc.dram_tensor("wve", [DC, DH], f32r, kind="ExternalInput")

    out = nc.dram_tensor("out", [RL, D], f32, kind="ExternalOutput")
    kc = nc.dram_tensor("kc", [B, HL, S, DC], f32, kind="ExternalOutput")
    vc = nc.dram_tensor("vc", [B, HL, S, DC], f32, kind="ExternalOutput")

    Exp = mybir.ActivationFunctionType.Exp

    with tile.TileContext(nc) as tc:
        with (
            tc.tile_pool(name="const", bufs=1) as CP,
            tc.tile_pool(name="persist", bufs=1) as PP,
            tc.tile_pool(name="dram", bufs=1, space="DRAM") as DP,
        ):
            ident = CP.tile([128, 128], f32)
            make_identity(nc, ident[:])
            # additive pre-exp causal mask for diagonal tiles:
            # maskpre[k, q] = 0 if q >= k else MASK_VAL
            maskpre = CP.tile([128, 128], f32)
            nc.gpsimd.memset(maskpre[:], 0.0)
            # keep 0 where q - k >= 0, else fill MASK_VAL
            nc.gpsimd.affine_select(
                out=maskpre[:], in_=maskpre[:],
                compare_op=mybir.AluOpType.is_ge,
                fill=MASK_VAL, base=0,
                pattern=[[1, 128]], channel_multiplier=-1,
            )
            ones_f = CP.tile([128, 1], f32)
            nc.gpsimd.memset(ones_f[:], 1.0)
            ones_col = CP.tile([128, 1], f32r)
            nc.scalar.copy(ones_col[:], ones_f[:])

            qT = [PP.tile([128, R], f32r, name=f"qT{h}") for h in range(HL)]
            khatT = [PP.tile([128, R], f32r, name=f"khatT{h}") for h in range(HL)]
            vhat = [PP.tile([128, R], f32r, name=f"vhat{h}") for h in range(HL)]

            # ---------------- phase 1: transpose + project + compress ----
            with (
                tc.tile_pool(name="ph1", bufs=1) as P1,
                tc.tile_pool(name="ph1ps", bufs=1, space="PSUM") as PS1,
            ):
                w_sb = {}
                for nm, wd in (("q", wq), ("k", wk), ("v", wv)):
                    t = P1.tile([128, KB * E], f32r, name=f"w{nm}sb")
                    nc.sync.dma_start(
                        t[:].rearrange("p (kb e) -> p kb e", kb=KB),
                        wd.ap().rearrange("(kb p) e -> p kb e", p=128))
                    w_sb[nm] = t
                wkc_sb = P1.tile([DH, DC], f32r)
                nc.sync.dma_start(wkc_sb[:], wkc.ap())
                wvc_sb = P1.tile([DH, DC], f32r)
                nc.sync.dma_start(wvc_sb[:], wvc.ap())
                wke_sb = P1.tile([DC, DH], f32r)
                nc.sync.dma_start(wke_sb[:], wke.ap())
                wve_sb = P1.tile([DC, DH], f32r)
                nc.sync.dma_start(wve_sb[:], wve.ap())

                for ch in range(NCH):
                    r0 = ch * CH
                    # load X rows [r0, r0+CH)
                    xin = []
                    for j in range(CH // 128):
                        t = P1.tile([128, D], f32, tag="xin", bufs=2,
                                    name=f"xin{ch}_{j}")
                        nc.sync.dma_start(
                            t[:], x.ap()[r0 + j * 128:r0 + (j + 1) * 128, :])
                        xin.append(t)
                    # transpose -> xt[kd] [128(d), CH(s)]
                    xt = []
                    for kd in range(KB):
                        t = P1.tile([128, CH], f32r, tag=f"xt{kd}", bufs=2,
                                    name=f"xt{ch}_{kd}")
                        xt.append(t)
                        for j in range(CH // 128):
                            tp = PS1.tile([128, 128], f32, tag="tp", bufs=2,
                                          name=f"tp{ch}_{kd}_{j}")
                            nc.tensor.transpose(
                                tp[:], xin[j][:, kd * 128:(kd + 1) * 128],
                                ident[:])
                            eng = nc.scalar if (kd + j) % 2 == 0 else nc.vector
                            if eng is nc.scalar:
                                nc.scalar.copy(
                                    t[:, j * 128:(j + 1) * 128], tp[:])
                            else:
                                nc.vector.tensor_copy(
                                    t[:, j * 128:(j + 1) * 128], tp[:])
                    # projections
                    kT = {}
                    for nm in ("q", "k", "v"):
                        for eb in range(HL):
                            pp_ = PS1.tile([128, CH], f32, tag="pp", bufs=2,
                                           name=f"pp{ch}_{nm}{eb}")
                            for kd in range(KB):
                                nc.tensor.matmul(
                                    pp_[:],
                                    w_sb[nm][:, kd * E + eb * 128:
                                             kd * E + eb * 128 + 128],
                                    xt[kd][:],
                                    start=(kd == 0), stop=(kd == KB - 1))
                            if nm == "q":
                                nc.scalar.copy(qT[eb][:, r0:r0 + CH], pp_[:])
                            else:
                                t = P1.tile([128, CH], f32r, tag=f"{nm}T{eb}",
                                            bufs=2, name=f"{nm}T{ch}_{eb}")
                                nc.scalar.copy(t[:], pp_[:])
                                kT[(nm, eb)] = t
                    # compression / expansion per head
                    for h in range(HL):
                        kT_h = kT[("k", h)]
                        vT_h = kT[("v", h)]
                        # k_cT [DC, CH]
                        pkc = PS1.tile([DC, CH], f32, tag="cps", bufs=3,
                                       name=f"pkc{ch}_{h}")
                        nc.tensor.matmul(pkc[:], wkc_sb[:], kT_h[:],
                                         start=True, stop=True)
                        kcT = P1.tile([DC, CH], f32r, tag="kcT", bufs=2,
                                      name=f"kcT{ch}_{h}")
                        nc.scalar.copy(kcT[:], pkc[:])
                        # k_hatT [128, CH] -> persistent
                        pkh = PS1.tile([128, CH], f32, tag="cps", bufs=3,
                                       name=f"pkh{ch}_{h}")
                        nc.tensor.matmul(pkh[:], wke_sb[:], kcT[:],
                                         start=True, stop=True)
                        nc.scalar.copy(khatT[h][:, r0:r0 + CH], pkh[:])
                        # v_cT [DC, CH]
                        pvc = PS1.tile([DC, CH], f32, tag="cps", bufs=3,
                                       name=f"pvc{ch}_{h}")
                        nc.tensor.matmul(pvc[:], wvc_sb[:], vT_h[:],
                                         start=True, stop=True)
                        vcT = P1.tile([DC, CH], f32r, tag="vcT", bufs=2,
                                      name=f"vcT{ch}_{h}")
                        nc.scalar.copy(vcT[:], pvc[:])
                        # v_hat [s, dh] per 128-row block -> persistent
                        for j in range(CH // 128):
                            gsb = ch * (CH // 128) + j
                            pvh = PS1.tile([128, DH], f32, tag="cps", bufs=3,
                                           name=f"pvh{ch}_{h}_{j}")
                            nc.tensor.matmul(
                                pvh[:], vcT[:, j * 128:(j + 1) * 128],
                                wve_sb[:], start=True, stop=True)
                            nc.scalar.copy(
                                vhat[h][:, gsb * 128:(gsb + 1) * 128], pvh[:])
                        # kc / vc outputs [s, DC]
                        for j in range(CH // 128):
                            gsb = ch * (CH // 128) + j
                            bi, sb_b = divmod(gsb, S // 128)
                            pko = PS1.tile([128, DC], f32, tag="cps", bufs=3,
                                           name=f"pko{ch}_{h}_{j}")
                            nc.tensor.matmul(
                                pko[:], kT_h[:, j * 128:(j + 1) * 128],
                                wkc_sb[:], start=True, stop=True)
                            ko = P1.tile([128, DC], f32, tag="ko", bufs=3,
                                         name=f"ko{ch}_{h}_{j}")
                            nc.vector.tensor_copy(ko[:], pko[:])
                            nc.sync.dma_start(
                                kc.ap()[bi, h, sb_b * 128:(sb_b + 1) * 128, :],
                                ko[:])
                            pvo = PS1.tile([128, DC], f32, tag="cps", bufs=3,
                                           name=f"pvo{ch}_{h}_{j}")
                            nc.tensor.matmul(
                                pvo[:], vT_h[:, j * 128:(j + 1) * 128],
                                wvc_sb[:], start=True, stop=True)
                            vo = P1.tile([128, DC], f32, tag="vo", bufs=3,
                                         name=f"vo{ch}_{h}_{j}")
                            nc.vector.tensor_copy(vo[:], pvo[:])
                            nc.sync.dma_start(
                                vc.ap()[bi, h, sb_b * 128:(sb_b + 1) * 128, :],
                                vo[:])

            # ---------------- phase 2: attention --------------------------
            a2a_in = DP.tile([N_CORES, HL, 128, QC], f32r, name="a2a_in")
            with (
                tc.tile_pool(name="ph2", bufs=1) as P2,
                tc.tile_pool(name="ph2ps", bufs=1, space="PSUM") as PS2,
            ):
                nctxT = [P2.tile([128, R], f32, name=f"nctxT{h}")
                         for h in range(HL)]
                for bi in range(B):
                    for h in range(HL):
                        for qc in range(NQC):
                            qlo = bi * S + qc * QC
                            nk = (qc + 1) * (QC // 128)
                            pctx = PS2.tile([128, QC], f32, tag="pctx",
                                            bufs=2, name=f"pctx{bi}_{h}_{qc}")
                            psum = PS2.tile([1, QC], f32, tag="psum", bufs=2,
                                            name=f"psum{bi}_{h}_{qc}")
                            for i in range(nk):
                                gb = bi * (S // 128) + i
                                off = max(0, 128 * (i - (QC // 128) * qc))
                                ps_s = PS2.tile([128, QC], f32, tag="ps",
                                                bufs=2,
                                                name=f"ps{bi}_{h}_{qc}_{i}")
                                nc.tensor.matmul(
                                    ps_s[:, off:],
                                    khatT[h][:, gb * 128:(gb + 1) * 128],
                                    qT[h][:, qlo + off:qlo + QC],
                                    start=True, stop=True)
                                if i >= (QC // 128) * qc:
                                    # diagonal tile: additive causal mask
                                    nc.vector.tensor_add(
                                        ps_s[:, off:off + 128],
                                        ps_s[:, off:off + 128], maskpre[:])
                                eT = P2.tile([128, QC], f32r, tag="eT",
                                             bufs=3, name=f"eT{bi}_{h}_{qc}_{i}")
                                nc.scalar.activation(
                                    eT[:, off:], ps_s[:, off:], Exp,
                                    scale=SCALE)
                                nc.tensor.matmul(
                                    psum[:, off:], ones_col[:], eT[:, off:],
                                    start=(i == 0), stop=(i == nk - 1))
                                nc.tensor.matmul(
                                    pctx[:, off:],
                                    vhat[h][:, gb * 128:(gb + 1) * 128],
                                    eT[:, off:],
                                    start=(i == 0), stop=(i == nk - 1))
                            rs = P2.tile([1, QC], f32, tag="rs", bufs=2,
                                         name=f"rs{bi}_{h}_{qc}")
                            nc.vector.reciprocal(rs[:], psum[:])
                            rb = P2.tile([128, QC], f32, tag="rb", bufs=2,
                                         name=f"rb{bi}_{h}_{qc}")
                            nc.gpsimd.partition_broadcast(rb[:], rs[:])
                            nc.vector.tensor_mul(
                                nctxT[h][:, qlo:qlo + QC], pctx[:], rb[:])
                # ship normalized ctxT to owners of each row slice
                for d in range(N_CORES):
                    for h in range(HL):
                        nc.sync.dma_start(
                            a2a_in[d, h, :, :],
                            nctxT[h][:, d * RL:(d + 1) * RL].bitcast(f32r))

            a2a_out = DP.tile([N_CORES, HL, 128, QC], f32r, name="a2a_out")
            nc.gpsimd.collective_compute(
                "AllToAll", mybir.AluOpType.bypass,
                replica_groups=[list(range(N_CORES))],
                ins=[a2a_in.opt()], outs=[a2a_out.opt()])

            # ---------------- phase 3: output projection -------------------
            with (
                tc.tile_pool(name="ph3", bufs=1) as P3,
                tc.tile_pool(name="ph3ps", bufs=1, space="PSUM") as PS3,
            ):
                gat = []
                for j in range(H):
                    t = P3.tile([128, RL], f32r, name=f"gat{j}")
                    nc.sync.dma_start(t[:], a2a_out[j // HL, j % HL, :, :])
                    gat.append(t)
                for ncx in range(D // 512):
                    wo_sb = P3.tile([128, KB * 512], f32r, tag="wo", bufs=2,
                                    name=f"wo{ncx}")
                    nc.sync.dma_start(
                        wo_sb[:].rearrange("p (kb e) -> p kb e", kb=KB),
                        wo.ap()[:, ncx * 512:(ncx + 1) * 512].rearrange(
                            "(kb p) e -> p kb e", p=128))
                    for m in range(RL // 128):
                        po = PS3.tile([128, 512], f32, tag="po", bufs=3,
                                      name=f"po{ncx}_{m}")
                        for kb in range(H):
                            nc.tensor.matmul(
                                po[:], gat[kb][:, m * 128:(m + 1) * 128],
                                wo_sb[:, kb * 512:(kb + 1) * 512],
                                start=(kb == 0), stop=(kb == H - 1))
                        ob = P3.tile([128, 512], f32, tag="ob", bufs=3,
                                     name=f"ob{ncx}_{m}")
                        nc.scalar.copy(ob[:], po[:])
                        nc.sync.dma_start(
                            out.ap()[m * 128:(m + 1) * 128,
                                     ncx * 512:(ncx + 1) * 512], ob[:])

    nc.compile()
    return nc


def _get_program():
    if "nc" not in _PROG:
        _PROG["nc"] = _build_program()
    return _PROG["nc"]


def kernel(hidden_states, Wq, Wk, Wv, Wo, Wkc, Wvc, Wke, Wve, **run_kwargs):
    nc = _get_program()

    x = np.ascontiguousarray(
        np.asarray(hidden_states, np.float32).reshape(R, D))
    Wq = np.asarray(Wq, np.float32)
    Wk = np.asarray(Wk, np.float32)
    Wv = np.asarray(Wv, np.float32)
    Wo = np.ascontiguousarray(np.asarray(Wo, np.float32))
    small = {
        "wkc": np.ascontiguousarray(np.asarray(Wkc, np.float32)),
        "wvc": np.ascontiguousarray(np.asarray(Wvc, np.float32)),
        "wke": np.ascontiguousarray(np.asarray(Wke, np.float32)),
        "wve": np.ascontiguousarray(np.asarray(Wve, np.float32)),
    }
    in_maps = []
    for c in range(N_CORES):
        sl = slice(E * c, E * (c + 1))
        in_maps.append({
            "x": x,
            "wq": np.ascontiguousarray(Wq[:, sl]),
            "wk": np.ascontiguousarray(Wk[:, sl]),
            "wv": np.ascontiguousarray(Wv[:, sl]),
            "wo": Wo,
            **small,
        })

    res = run_bass_kernel_spmd(nc, in_maps, list(range(N_CORES)), **run_kwargs)

    attn = np.empty((R, D), np.float32)
    kc_full = np.empty((B, H, S, DC), np.float32)
    vc_full = np.empty((B, H, S, DC), np.float32)
    for c in range(N_CORES):
        attn[RL * c:RL * (c + 1)] = res.results[c]["out"]
        kc_full[:, HL * c:HL * (c + 1)] = res.results[c]["kc"]
        vc_full[:, HL * c:HL * (c + 1)] = res.results[c]["vc"]
    outs = (attn.reshape(B, S, D), kc_full, vc_full)
    if run_kwargs:
        return outs, res
    return outs


# revision 11
# speedup vs baseline: 1.1481x; 1.1481x over previous
"""Trainium2 Bass kernel for CompressedAttentionWrapper.

Sharding: tensor-parallel over heads across 8 NeuronCores (2 heads/core).
Wq/Wk/Wv column-split by head; per-head compressors and compressed cache
local to each core. The output projection redistributes the per-head
attention context with an on-chip AllToAll so each core computes a 512-row
slice of the final output against the full Wo.

Per-core dataflow (all big matmuls in float32r at full PE rate):
  phase 1: stream X in 256-row chunks, PE-transpose to X^T, project to
           qT/kT/vT (dh on partitions), chain the K/V compression
           (k_c = k@Wkc) and expansion (k_hat = k_c@Wke) per head; emit
           k_compressed / v_compressed output tiles on the way.
  phase 2: causal attention per (batch, head) on transposed scores:
           scoresT = k_hatT.T-blocks @ qT, exp on ACT (scale fused), causal
           handled structurally (block skipping + additive diag mask),
           row-sums via ones-matmul, ctxT accumulated in PSUM, normalized
           with reciprocal broadcast.
  phase 3: AllToAll of normalized ctxT (4MB/core), then out-rows slice =
           gathered-ctxT.T @ full Wo.
"""

import sys

for _p in ("/opt/trn_rl_repo", "/opt/pypackages"):
    if _p not in sys.path:
        sys.path.append(_p)

import numpy as np
import concourse.bass as bass  # noqa: F401  (import keeps bass registered)
import concourse.mybir as mybir
import concourse.tile as tile
from concourse import bacc
from concourse.bass_utils import run_bass_kernel_spmd
from concourse.masks import make_identity

B, S, D = 2, 2048, 2048
H, DH, DC = 16, 128, 32
N_CORES = 8
HL = H // N_CORES           # heads per core (2)
E = HL * DH                 # local projection width (256)
R = B * S                   # flattened rows (4096)
RL = R // N_CORES           # output rows per core (512)
CH = 256                    # phase-1 row chunk
NCH = R // CH
QC = 512                    # phase-2 query chunk
NQC = S // QC
KB = D // 128               # contraction blocks (16)
SCALE = 1.0 / float(np.sqrt(DH))
MASK_VAL = -1.0e5

f32 = mybir.dt.float32
f32r = mybir.dt.float32r

_PROG = {}


def _build_program():
    nc = bacc.Bacc("TRN2", target_bir_lowering=False, debug=False,
                   num_devices=N_CORES)

    x = nc.dram_tensor("x", [R, D], f32, kind="ExternalInput")
    wq = nc.dram_tensor("wq", [D, E], f32r, kind="ExternalInput")
    wk = nc.dram_tensor("wk", [D, E], f32r, kind="ExternalInput")
    wv = nc.dram_tensor("wv", [D, E], f32r, kind="ExternalInput")
    wo = nc.dram_tensor("wo", [D, D], f32r, kind="ExternalInput")
    wkc = nc.dram_tensor("wkc", [DH, DC], f32r, kind="ExternalInput")
    wvc = nc.dram_tensor("wvc", [DH, DC], f32r, kind="ExternalInput")
    wke = nc.dram_tensor("wke", [DC, DH], f32r, kind="ExternalInput")
    wve = nc.dram_tensor("wve", [DC, DH], f32r, kind="ExternalInput")

    out = nc.dram_tensor("out", [RL, D], f32, kind="ExternalOutput")
    kc = nc.dram_tensor("kc", [B, HL, S, DC], f32, kind="ExternalOutput")
    vc = nc.dram_tensor("vc", [B, HL, S, DC], f32, kind="ExternalOutput")

    Exp = mybir.ActivationFunctionType.Exp

    with tile.TileContext(nc) as tc:
        with (
            tc.tile_pool(name="const", bufs=1) as CP,
            tc.tile_pool(name="persist", bufs=1) as PP,
            tc.tile_pool(name="dram", bufs=1, space="DRAM") as DP,
        ):
            ident = CP.tile([128, 128], f32)
            make_identity(nc, ident[:])
            # additive pre-exp causal mask for diagonal tiles:
            # maskpre[k, q] = 0 if q >= k else MASK_VAL
            maskpre = CP.tile([128, 128], f32)
            nc.gpsimd.memset(maskpre[:], 0.0)
            # keep 0 where q - k >= 0, else fill MASK_VAL
            nc.gpsimd.affine_select(
                out=maskpre[:], in_=maskpre[:],
                compare_op=mybir.AluOpType.is_ge,
                fill=MASK_VAL, base=0,
                pattern=[[1, 128]], channel_multiplier=-1,
            )
            ones_f = CP.tile([128, 1], f32)
            nc.gpsimd.memset(ones_f[:], 1.0)
            ones_col = CP.tile([128, 1], f32r)
            nc.scalar.copy(ones_col[:], ones_f[:])

            qT = [PP.tile([128, R], f32r, name=f"qT{h}") for h in range(HL)]
            khatT = [PP.tile([128, R], f32r, name=f"khatT{h}") for h in range(HL)]
            vhat = [PP.tile([128, R], f32r, name=f"vhat{h}") for h in range(HL)]

            # ---------------- phase 1: transpose + project + compress ----
            with (
                tc.tile_pool(name="ph1", bufs=1) as P1,
                tc.tile_pool(name="ph1ps", bufs=1, space="PSUM") as PS1,
            ):
                w_sb = {}
                for nm, wd in (("q", wq), ("k", wk), ("v", wv)):
                    t = P1.tile([128, KB * E], f32r, name=f"w{nm}sb")
                    nc.sync.dma_start(
                        t[:].rearrange("p (kb e) -> p kb e", kb=KB),
                        wd.ap().rearrange("(kb p) e -> p kb e", p=128))
                    w_sb[nm] = t
                wkc_sb = P1.tile([DH, DC], f32r)
                nc.sync.dma_start(wkc_sb[:], wkc.ap())
                wvc_sb = P1.tile([DH, DC], f32r)
                nc.sync.dma_start(wvc_sb[:], wvc.ap())
                wke_sb = P1.tile([DC, DH], f32r)
                nc.sync.dma_start(wke_sb[:], wke.ap())
                wve_sb = P1.tile([DC, DH], f32r)
                nc.sync.dma_start(wve_sb[:], wve.ap())

                for ch in range(NCH):
                    r0 = ch * CH
                    # load X rows [r0, r0+CH)
                    xin = []
                    for j in range(CH // 128):
                        t = P1.tile([128, D], f32, tag="xin", bufs=2,
                                    name=f"xin{ch}_{j}")
                        nc.sync.dma_start(
                            t[:], x.ap()[r0 + j * 128:r0 + (j + 1) * 128, :])
                        xin.append(t)
                    # transpose -> xt[kd] [128(d), CH(s)]
                    xt = []
                    for kd in range(KB):
                        t = P1.tile([128, CH], f32r, tag=f"xt{kd}", bufs=2,
                                    name=f"xt{ch}_{kd}")
                        xt.append(t)
                        for j in range(CH // 128):
                            tp = PS1.tile([128, 128], f32, tag="tp", bufs=3,
                                          name=f"tp{ch}_{kd}_{j}")
                            nc.tensor.transpose(
                                tp[:], xin[j][:, kd * 128:(kd + 1) * 128],
                                ident[:])
                            eng = nc.scalar if (kd + j) % 2 == 0 else nc.vector
                            if eng is nc.scalar:
                                nc.scalar.copy(
                                    t[:, j * 128:(j + 1) * 128], tp[:])
                            else:
                                nc.vector.tensor_copy(
                                    t[:, j * 128:(j + 1) * 128], tp[:])
                    # projections
                    kT = {}
                    for nm in ("q", "k", "v"):
                        for eb in range(HL):
                            pp_ = PS1.tile([128, CH], f32, tag="pp", bufs=2,
                                           name=f"pp{ch}_{nm}{eb}")
                            for kd in range(KB):
                                nc.tensor.matmul(
                                    pp_[:],
                                    w_sb[nm][:, kd * E + eb * 128:
                                             kd * E + eb * 128 + 128],
                                    xt[kd][:],
                                    start=(kd == 0), stop=(kd == KB - 1))
                            if nm == "q":
                                nc.scalar.copy(qT[eb][:, r0:r0 + CH], pp_[:])
                            else:
                                t = P1.tile([128, CH], f32r, tag=f"{nm}T{eb}",
                                            bufs=2, name=f"{nm}T{ch}_{eb}")
                                nc.scalar.copy(t[:], pp_[:])
                                kT[(nm, eb)] = t
                    # compression / expansion per head
                    for h in range(HL):
                        kT_h = kT[("k", h)]
                        vT_h = kT[("v", h)]
                        # k_cT [DC, CH]
                        pkc = PS1.tile([DC, CH], f32, tag="cps", bufs=3,
                                       name=f"pkc{ch}_{h}")
                        nc.tensor.matmul(pkc[:], wkc_sb[:], kT_h[:],
                                         start=True, stop=True)
                        kcT = P1.tile([DC, CH], f32r, tag="kcT", bufs=2,
                                      name=f"kcT{ch}_{h}")
                        nc.scalar.copy(kcT[:], pkc[:])
                        # k_hatT [128, CH] -> persistent
                        pkh = PS1.tile([128, CH], f32, tag="cps", bufs=3,
                                       name=f"pkh{ch}_{h}")
                        nc.tensor.matmul(pkh[:], wke_sb[:], kcT[:],
                                         start=True, stop=True)
                        nc.scalar.copy(khatT[h][:, r0:r0 + CH], pkh[:])
                        # v_cT [DC, CH]
                        pvc = PS1.tile([DC, CH], f32, tag="cps", bufs=3,
                                       name=f"pvc{ch}_{h}")
                        nc.tensor.matmul(pvc[:], wvc_sb[:], vT_h[:],
                                         start=True, stop=True)
                        vcT = P1.tile([DC, CH], f32r, tag="vcT", bufs=2,
                                      name=f"vcT{ch}_{h}")
                        nc.scalar.copy(vcT[:], pvc[:])
                        # v_hat [s, dh] per 128-row block -> persistent
                        for j in range(CH // 128):
                            gsb = ch * (CH // 128) + j
                            pvh = PS1.tile([128, DH], f32, tag="cps", bufs=3,
                                           name=f"pvh{ch}_{h}_{j}")
                            nc.tensor.matmul(
                                pvh[:], vcT[:, j * 128:(j + 1) * 128],
                                wve_sb[:], start=True, stop=True)
                            nc.scalar.copy(
                                vhat[h][:, gsb * 128:(gsb + 1) * 128], pvh[:])
                        # kc / vc outputs [s, DC]
                        for j in range(CH // 128):
                            gsb = ch * (CH // 128) + j
                            bi, sb_b = divmod(gsb, S // 128)
                            pko = PS1.tile([128, DC], f32, tag="cps", bufs=3,
                                           name=f"pko{ch}_{h}_{j}")
                            nc.tensor.matmul(
                                pko[:], kT_h[:, j * 128:(j + 1) * 128],
                                wkc_sb[:], start=True, stop=True)
                            ko = P1.tile([128, DC], f32, tag="ko", bufs=3,
                                         name=f"ko{ch}_{h}_{j}")
                            nc.vector.tensor_copy(ko[:], pko[:])
                            nc.sync.dma_start(
                                kc.ap()[bi, h, sb_b * 128:(sb_b + 1) * 128, :],
                                ko[:])
                            pvo = PS1.tile([128, DC], f32, tag="cps", bufs=3,
                                           name=f"pvo{ch}_{h}_{j}")
                            nc.tensor.matmul(
                                pvo[:], vT_h[:, j * 128:(j + 1) * 128],
                                wvc_sb[:], start=True, stop=True)
                            vo = P1.tile([128, DC], f32, tag="vo", bufs=3,
                                         name=f"vo{ch}_{h}_{j}")
                            nc.vector.tensor_copy(vo[:], pvo[:])
                            nc.sync.dma_start(
                                vc.ap()[bi, h, sb_b * 128:(sb_b + 1) * 128, :],
                                vo[:])

            # ---------------- phase 2: attention --------------------------
            a2a_in = [DP.tile([N_CORES, 128, QC], f32r, name=f"a2a_in{h}")
                      for h in range(HL)]
            a2a_out = [DP.tile([N_CORES, 128, QC], f32r, name=f"a2a_out{h}")
                       for h in range(HL)]
            with (
                tc.tile_pool(name="ph2", bufs=1) as P2,
                tc.tile_pool(name="ph2ps", bufs=1, space="PSUM") as PS2,
            ):
                nctxT = [P2.tile([128, R], f32, name=f"nctxT{h}")
                         for h in range(HL)]
                for h in range(HL):
                    for bi in range(B):
                        for qc in range(NQC):
                            qlo = bi * S + qc * QC
                            nk = (qc + 1) * (QC // 128)
                            pctx = PS2.tile([128, QC], f32, tag="pctx",
                                            bufs=2, name=f"pctx{bi}_{h}_{qc}")
                            psum = PS2.tile([1, QC], f32, tag="psum", bufs=2,
                                            name=f"psum{bi}_{h}_{qc}")
                            for i in range(nk):
                                gb = bi * (S // 128) + i
                                off = max(0, 128 * (i - (QC // 128) * qc))
                                ps_s = PS2.tile([128, QC], f32, tag="ps",
                                                bufs=3,
                                                name=f"ps{bi}_{h}_{qc}_{i}")
                                nc.tensor.matmul(
                                    ps_s[:, off:],
                                    khatT[h][:, gb * 128:(gb + 1) * 128],
                                    qT[h][:, qlo + off:qlo + QC],
                                    start=True, stop=True)
                                if i >= (QC // 128) * qc:
                                    # diagonal tile: additive causal mask
                                    nc.vector.tensor_add(
                                        ps_s[:, off:off + 128],
                                        ps_s[:, off:off + 128], maskpre[:])
                                eT = P2.tile([128, QC], f32r, tag="eT",
                                             bufs=4, name=f"eT{bi}_{h}_{qc}_{i}")
                                nc.scalar.activation(
                                    eT[:, off:], ps_s[:, off:], Exp,
                                    scale=SCALE)
                                nc.tensor.matmul(
                                    psum[:, off:], ones_col[:], eT[:, off:],
                                    start=(i == 0), stop=(i == nk - 1))
                                nc.tensor.matmul(
                                    pctx[:, off:],
                                    vhat[h][:, gb * 128:(gb + 1) * 128],
                                    eT[:, off:],
                                    start=(i == 0), stop=(i == nk - 1))
                            rs = P2.tile([1, QC], f32, tag="rs", bufs=2,
                                         name=f"rs{bi}_{h}_{qc}")
                            nc.vector.reciprocal(rs[:], psum[:])
                            rb = P2.tile([128, QC], f32, tag="rb", bufs=2,
                                         name=f"rb{bi}_{h}_{qc}")
                            nc.gpsimd.partition_broadcast(rb[:], rs[:])
                            nc.vector.tensor_mul(
                                nctxT[h][:, qlo:qlo + QC], pctx[:], rb[:])
                    # ship this head's normalized ctxT to each row-slice
                    # owner; the collective overlaps the next head's
                    # attention (and phase 3 for the last head).
                    for d in range(N_CORES):
                        nc.sync.dma_start(
                            a2a_in[h][d, :, :],
                            nctxT[h][:, d * RL:(d + 1) * RL].bitcast(f32r))
                    nc.gpsimd.collective_compute(
                        "AllToAll", mybir.AluOpType.bypass,
                        replica_groups=[list(range(N_CORES))],
                        ins=[a2a_in[h].opt()], outs=[a2a_out[h].opt()])

            # ---------------- phase 3: output projection -------------------
            with (
                tc.tile_pool(name="ph3", bufs=1) as P3,
                tc.tile_pool(name="ph3ps", bufs=1, space="PSUM") as PS3,
            ):
                # even global heads arrive with the first AllToAll; load and
                # consume them first so phase 3 overlaps the second one.
                korder = list(range(0, H, 2)) + list(range(1, H, 2))
                gat = {}
                for j in korder:
                    t = P3.tile([128, RL], f32r, name=f"gat{j}")
                    nc.sync.dma_start(t[:], a2a_out[j % HL][j // HL, :, :])
                    gat[j] = t
                for ncx in range(D // 512):
                    wo_sb = P3.tile([128, KB * 512], f32r, tag="wo", bufs=2,
                                    name=f"wo{ncx}")
                    nc.sync.dma_start(
                        wo_sb[:].rearrange("p (kb e) -> p kb e", kb=KB),
                        wo.ap()[:, ncx * 512:(ncx + 1) * 512].rearrange(
                            "(kb p) e -> p kb e", p=128))
                    for m in range(RL // 128):
                        po = PS3.tile([128, 512], f32, tag="po", bufs=3,
                                      name=f"po{ncx}_{m}")
                        for ki, kb in enumerate(korder):
                            nc.tensor.matmul(
                                po[:], gat[kb][:, m * 128:(m + 1) * 128],
                                wo_sb[:, kb * 512:(kb + 1) * 512],
                                start=(ki == 0), stop=(ki == H - 1))
                        ob = P3.tile([128, 512], f32, tag="ob", bufs=3,
                                     name=f"ob{ncx}_{m}")
                        nc.scalar.copy(ob[:], po[:])
                        nc.sync.dma_start(
                            out.ap()[m * 128:(m + 1) * 128,
                                     ncx * 512:(ncx + 1) * 512], ob[:])

    nc.compile()
    return nc


def _get_program():
    if "nc" not in _PROG:
        _PROG["nc"] = _build_program()
    return _PROG["nc"]


def kernel(hidden_states, Wq, Wk, Wv, Wo, Wkc, Wvc, Wke, Wve, **run_kwargs):
    nc = _get_program()

    x = np.ascontiguousarray(
        np.asarray(hidden_states, np.float32).reshape(R, D))
    Wq = np.asarray(Wq, np.float32)
    Wk = np.asarray(Wk, np.float32)
    Wv = np.asarray(Wv, np.float32)
    Wo = np.ascontiguousarray(np.asarray(Wo, np.float32))
    small = {
        "wkc": np.ascontiguousarray(np.asarray(Wkc, np.float32)),
        "wvc": np.ascontiguousarray(np.asarray(Wvc, np.float32)),
        "wke": np.ascontiguousarray(np.asarray(Wke, np.float32)),
        "wve": np.ascontiguousarray(np.asarray(Wve, np.float32)),
    }
    in_maps = []
    for c in range(N_CORES):
        sl = slice(E * c, E * (c + 1))
        in_maps.append({
            "x": x,
            "wq": np.ascontiguousarray(Wq[:, sl]),
            "wk": np.ascontiguousarray(Wk[:, sl]),
            "wv": np.ascontiguousarray(Wv[:, sl]),
            "wo": Wo,
            **small,
        })

    res = run_bass_kernel_spmd(nc, in_maps, list(range(N_CORES)), **run_kwargs)

    attn = np.empty((R, D), np.float32)
    kc_full = np.empty((B, H, S, DC), np.float32)
    vc_full = np.empty((B, H, S, DC), np.float32)
    for c in range(N_CORES):
        attn[RL * c:RL * (c + 1)] = res.results[c]["out"]
        kc_full[:, HL * c:HL * (c + 1)] = res.results[c]["kc"]
        vc_full[:, HL * c:HL * (c + 1)] = res.results[c]["vc"]
    outs = (attn.reshape(B, S, D), kc_full, vc_full)
    if run_kwargs:
        return outs, res
    return outs
